# revision 11
# baseline (speedup 1.0000x reference)
"""Trainium2 Bass kernel for nn_Classifier_42588895707508 (fp16 fold version).

Computation (see reference):
    pool_k[b, h] = max_{s < eff_k[b]} x_k[b, s, h]      (k = 1, 2)
    out[b, c]    = sum_h pool_1[b,h] W[c,h] + pool_2[b,h] W[c, 768+h] + bias[c]
where eff_k[b] is derived from the mask m_k (index of first zero; 0 -> S).

Strategy (memory-bound, ragged sequences):
  * Only the valid prefix of each sample row touches the device, packed
    densely per core (transposed: h%128 on partitions, per slot 6 h-chunks
    x width contiguous), in FP16 -- halving HBM traffic vs fp32. The 2e-2
    harness gate leaves ~100x margin for fp16 rounding.
  * Rows (kind, sample) are distributed round-robin by length rank across
    the 8 cores -> identical SPMD program, balanced load.
  * Pooling runs on the DVE as tensor-tensor max folds
    (tensor_tensor with op=max), which in 16-bit
    get the 2x_1p perf mode (2 elem/lane/cyc) -- unlike tensor_reduce
    which is stuck at 1x. Each fold halves a group's width with an
    overlapped (idempotent, max) split so every level keeps 4B alignment
    with only pad-to-multiple-of-4.
  * Folds stop at width 16 writing into a persistent staging tile
    [128, 128slots*6ch, 16]; shared folds (16->8->4->2->1) finish 32
    slots at a time, then the tiny linear layer runs on the tensor
    engine in fp16 (K=128 per chunk, PSUM f32).
  * Groups (equal padded width) are planned by a small DP balancing DMA
    padding vs DVE instruction count; DMA tiles ramp up at the start and
    taper at the end so the vector engine starts early and drains fast.
"""

import numpy as np

B, S, H, C = 512, 256, 768, 2
NCORES = 8
CH = H // 128            # 6 h-chunks of 128 partitions
KINDS = 2
SLOTS = B // NCORES      # 64 slots per kind per core
SLOT_TOT = KINDS * SLOTS
NEG = np.float16(-60000.0)

TILE_W = 20480           # fp16 elems per partition per SBUF data tile
DATA_BUFS = 4
SEG_RAMP = [512, 1024, 2048, 4096, 6144]
SEG_W = 9216             # steady-state dma segment width

# planning cost model
DMA_NS_PER_COL = 1536.0 / 410.0   # one slot-col = 6 elems x 128 part x 2B
DVE_GHZ = 0.96
OP_OVERHEAD_CYC = 105.0  # measured ~110ns/op incl drain
LAMBDAS = [0.5, 1.0, 1.5, 2.0, 3.0, 4.0, 6.0, 8.0, 12.0]
STW = 16                 # staging width: fold chains stop here


def _eff_lengths(m):
    am = np.argmin(np.asarray(m), axis=1)
    return np.where(am == 0, S, am).astype(np.int64)


def _ceil4(x):
    return (int(x) + 3) // 4 * 4


def _chain(W):
    """Fold chain for a group of width W (mult of 4, >= 16).

    Returns (n_ops, out_elems_per_row): in-place halving folds down to
    <= 32 wide, then one fold to width 16 (into staging); the shared
    epilogue finishes 16 -> 1."""
    ops, outs, c = 0, 0, int(W)
    while c > 2 * STW:
        h = _ceil4((c + 1) // 2)
        ops += 1
        outs += h
        c = h
    ops += 1
    outs += STW
    return ops, outs


def _plan_widths(widths, lam):
    """Group descending padded widths into equal-width groups via DP.

    Cost per group: DMA of n*W slot-cols + lam * per-op overhead.
    Constraint: n*CH*W <= TILE_W. Returns final width per slot."""
    n = len(widths)
    INF = float("inf")
    best = [INF] * (n + 1)
    best[0] = 0.0
    prev = [0] * (n + 1)
    for i in range(1, n + 1):
        j = i - 1
        while j >= 0:
            Wg = widths[j]
            if (i - j) * CH * Wg > TILE_W:
                break
            ops, _ = _chain(Wg)
            cost = (
                best[j]
                + (i - j) * Wg * DMA_NS_PER_COL
                + lam * ops * OP_OVERHEAD_CYC / DVE_GHZ
            )
            if cost < best[i]:
                best[i] = cost
                prev[i] = j
            j -= 1
    out = np.zeros(n, dtype=np.int64)
    i = n
    while i > 0:
        j = prev[i]
        out[j:i] = widths[j]
        i = j
    return out


def _estimate(final_widths_by_kind):
    """(dma_ns, dve_ns) rough estimates for a plan."""
    cols = 0
    work_cyc = 0.0
    ops = 0
    for fw in final_widths_by_kind:
        i = 0
        n = len(fw)
        while i < n:
            j = i
            while j < n and fw[j] == fw[i] and (j - i + 1) * CH * fw[i] <= TILE_W:
                j += 1
            g = j - i
            c_ops, c_outs = _chain(fw[i])
            cols += g * fw[i]
            work_cyc += CH * g * c_outs / 2.0
            ops += c_ops
            i = j
    dma_ns = cols * DMA_NS_PER_COL
    dve_ns = (work_cyc + ops * OP_OVERHEAD_CYC + 9000.0) / DVE_GHZ
    return dma_ns, dve_ns


def _make_plan(slot_w):
    """slot_w: [kind][64] true widths desc. Returns final padded widths."""
    padded = [np.maximum([_ceil4(w) for w in sw], STW) for sw in slot_w]
    best = None
    for lam in LAMBDAS:
        fw = [_plan_widths(p, lam) for p in padded]
        dma, dve = _estimate(fw)
        # wall model: DVE path (starts ~10.5us after t0) vs DMA path with
        # ~19% slow-engine margin (engine 15 on unlucky cores)
        obj = (max(dve + 10500.0, dma * 1.19 + 8500.0), dma)
        if best is None or obj < best[0]:
            best = (obj, fw)
    return best[1]


def _layout(final_w):
    """Row-granular layout from final slot widths.

    Each slot is 6 chunk-rows of width W, contiguous in DRAM columns and
    in staging rows. Returns (R, tiles); tiles = [(c0, c1, segments)],
    segments = [(s0, s1, [(a, row0, nrows, W), ...])] where a is the
    elem offset of the fold-group, row0 its first staging row.
    """
    row_w = []  # width per staging row, in emission order
    for k in range(KINDS):
        for i in range(SLOTS):
            row_w.extend([int(final_w[k][i])] * CH)
    total = sum(row_w)
    offs = [np.zeros(SLOTS, dtype=np.int64) for _ in range(KINDS)]
    off = 0
    for k in range(KINDS):
        for i in range(SLOTS):
            offs[k][i] = off
            off += CH * int(final_w[k][i])

    # segment schedule: ramp up, steady, taper down
    seg_caps = []
    done = 0
    nrows = len(row_w)
    r = 0
    segs_flat = []  # (elem_start, elem_end, row_start, row_end)
    off = 0
    while r < nrows:
        rem = total - done
        nseg = len(segs_flat)
        if nseg < len(SEG_RAMP):
            cap = SEG_RAMP[nseg]
        elif rem <= 12288:
            cap = max(2048, rem // 3 + 256)
        else:
            cap = SEG_W
        e0 = off
        r0 = r
        cur = 0
        while r < nrows and (cur == 0 or cur + row_w[r] <= cap):
            cur += row_w[r]
            off += row_w[r]
            r += 1
        done += cur
        segs_flat.append((e0, off, r0, r))

    # pack segments into data tiles (SBUF buffers) of <= TILE_W elems
    tiles = []
    cur_segs = []
    for (e0, e1, r0, r1) in segs_flat:
        if cur_segs and e1 - cur_segs[0][0] > TILE_W:
            tiles.append(cur_segs)
            cur_segs = []
        cur_segs.append((e0, e1, r0, r1))
    if cur_segs:
        tiles.append(cur_segs)

    out_tiles = []
    for segs in tiles:
        c0 = segs[0][0]
        c1 = segs[-1][1]
        out_segs = []
        for (e0, e1, r0, r1) in segs:
            groups = []
            a = e0
            for r in range(r0, r1):
                if groups and groups[-1][3] == row_w[r]:
                    groups[-1][2] += 1
                else:
                    groups.append([a, r, 1, row_w[r]])
                a += row_w[r]
            out_segs.append((e0, e1, tuple(tuple(g) for g in groups)))
        out_tiles.append((c0, c1, tuple(out_segs)))
    return total, tuple(out_tiles), offs


def _build_program(R, tiles):
    import concourse.bacc as bacc
    import concourse.mybir as mybir
    from concourse.tile import TileContext

    f16 = mybir.dt.float16
    f32 = mybir.dt.float32
    MAX = mybir.AluOpType.max

    # last tile index touching each 16-slot staging chunk, for epilogues
    NHALF = 8
    HROWS = SLOT_TOT * CH // NHALF
    last_tile_of_half = [0] * NHALF
    for ti, (_, _, segs) in enumerate(tiles):
        for (_, _, groups) in segs:
            for (_, r0, nr, _) in groups:
                for h in range(r0 // HROWS, (r0 + nr - 1) // HROWS + 1):
                    last_tile_of_half[h] = ti

    nc = bacc.Bacc("TRN2", target_bir_lowering=False, debug=False, num_devices=NCORES)
    p_in = nc.dram_tensor("p", [128, R], f16, kind="ExternalInput")
    wt_in = nc.dram_tensor("wt", [128, KINDS * CH, C], f16, kind="ExternalInput")
    out_d = nc.dram_tensor("out", [C, SLOT_TOT], f32, kind="ExternalOutput")

    with TileContext(nc) as tc:
        with (
            tc.tile_pool(name="data", bufs=DATA_BUFS) as data_pool,
            tc.tile_pool(name="stage", bufs=1) as stage_pool,
            tc.tile_pool(name="small", bufs=1) as small_pool,
            tc.tile_pool(name="psum", bufs=1, space="PSUM") as psum_pool,
        ):
            wt_t = small_pool.tile([128, KINDS * CH, C], f16, tag="wt")
            stage = stage_pool.tile([128, SLOT_TOT * CH, STW], f16, tag="stage")
            out_sb = small_pool.tile([C, SLOT_TOT], f32, tag="osb")

            def fold(view, w, h, out_view=None):
                # out = max(view[:, :, :h], view[:, :, w-h:w]) (overlap ok)
                nc.vector.tensor_tensor(
                    out=view[:, :, :h] if out_view is None else out_view,
                    in0=view[:, :, :h],
                    in1=view[:, :, w - h : w],
                    op=MAX,
                )

            def epilogue_half(hf):
                rows = stage[:, hf * HROWS : (hf + 1) * HROWS, :]
                w = STW
                while w > 1:
                    fold(rows, w, w // 2)
                    w //= 2
                if hf % (NHALF // KINDS) != NHALF // KINDS - 1:
                    return
                k = hf // (NHALF // KINDS)
                srg = stage[:, :, :].rearrange("p (s c) e -> p s c e", c=CH)
                ps = psum_pool.tile([C, SLOTS], f32, tag=f"ps{k}")
                for ch in range(CH):
                    nc.tensor.matmul(
                        ps,
                        lhsT=wt_t[:, k * CH + ch, :],
                        rhs=srg[:, k * SLOTS : (k + 1) * SLOTS, ch, 0],
                        start=(ch == 0),
                        stop=(ch == CH - 1),
                    )
                nc.scalar.copy(out=out_sb[:, k * SLOTS : (k + 1) * SLOTS], in_=ps)
                nc.sync.dma_start(
                    out=out_d[:, k * SLOTS : (k + 1) * SLOTS],
                    in_=out_sb[:, k * SLOTS : (k + 1) * SLOTS],
                )

            nseg_done = 0
            for ti, (c0, c1, segs) in enumerate(tiles):
                dt = data_pool.tile([128, TILE_W], f16, tag="data")
                for (e0, e1, groups) in segs:
                    nc.sync.dma_start(
                        out=dt[:, e0 - c0 : e1 - c0], in_=p_in[:, e0:e1]
                    )
                    nseg_done += 1
                    if nseg_done == 2:
                        nc.scalar.dma_start(out=wt_t, in_=wt_in[:, :, :])
                    for (a, r0, nr, W) in groups:
                        base = a - c0
                        view = dt[:, base : base + nr * W].rearrange(
                            "p (r w) -> p r w", w=W
                        )
                        w = W
                        while w > 2 * STW:
                            h = _ceil4((w + 1) // 2)
                            fold(view, w, h)
                            w = h
                        st = stage[:, r0 : r0 + nr, :]
                        fold(view, w, STW, out_view=st)
                for hf in range(NHALF):
                    if last_tile_of_half[hf] == ti:
                        epilogue_half(hf)

    nc.compile()
    return nc


_NC_CACHE = {}


def kernel(x1, x2, m1, m2, W, b, _run_opts=None):
    from concourse.bass_utils import run_bass_kernel_spmd

    x1 = np.asarray(x1)
    x2 = np.asarray(x2)
    W32 = np.asarray(W, dtype=np.float32)
    b32 = np.asarray(b, dtype=np.float32)
    effs = [_eff_lengths(m1), _eff_lengths(m2)]
    orders = [np.argsort(-effs[k], kind="stable") for k in range(KINDS)]
    slot_w = [effs[k][orders[k][:: NCORES]].astype(np.int64) for k in range(KINDS)]

    final_w = _make_plan(slot_w)
    R, tiles, offs = _layout(final_w)

    key = (R, tuple(tiles))
    nc = _NC_CACHE.get(key)
    if nc is None:
        nc = _build_program(R, tiles)
        _NC_CACHE[key] = nc

    # pack per-core fp16 data
    xh = [x1.astype(np.float16), x2.astype(np.float16)]
    packs = np.full((NCORES, 128, R), NEG, dtype=np.float16)
    for k in range(KINDS):
        eff, order = effs[k], orders[k]
        xk = xh[k]
        for i in range(SLOTS):
            off = int(offs[k][i])
            v = int(final_w[k][i])
            for c in range(NCORES):
                bidx = order[i * NCORES + c]
                e = int(eff[bidx])
                dst = packs[c][:, off : off + CH * v].reshape(128, CH, v)
                dst[:, :, :e] = xk[bidx, :e, :].reshape(e, CH, 128).transpose(2, 1, 0)

    wtp = np.ascontiguousarray(
        W32.astype(np.float16).reshape(C, KINDS, CH, 128).transpose(3, 1, 2, 0)
    ).reshape(128, KINDS * CH, C)

    in_maps = [{"p": packs[c], "wt": wtp} for c in range(NCORES)]

    res = None
    last_err = None
    for _attempt in range(3):
        try:
            res = run_bass_kernel_spmd(
                nc, in_maps, core_ids=list(range(NCORES)), **(_run_opts or {})
            )
            break
        except Exception as e:  # wedged device etc. -- retry
            last_err = e
    if res is None:
        raise last_err

    out_full = np.zeros((B, C), dtype=np.float32)
    res_all = np.stack([res.results[c]["out"] for c in range(NCORES)])  # [8, C, 128]
    for k in range(KINDS):
        part = res_all[:, :, k * SLOTS : (k + 1) * SLOTS]  # [core, C, slot]
        part = part.transpose(2, 0, 1).reshape(B, C)  # [(slot, core), C]
        out_full[orders[k]] += part
    out_full += b32[None, :]
    if _run_opts is not None:
        kernel._last_res = res
    return out_full


# revision 15
# speedup vs baseline: 1.0218x; 1.0218x over previous
"""Trainium2 Bass kernel for nn_Classifier_42588895707508 (fp16 fold version).

Computation (see reference):
    pool_k[b, h] = max_{s < eff_k[b]} x_k[b, s, h]      (k = 1, 2)
    out[b, c]    = sum_h pool_1[b,h] W[c,h] + pool_2[b,h] W[c, 768+h] + bias[c]
where eff_k[b] is derived from the mask m_k (index of first zero; 0 -> S).

Strategy (memory-bound, ragged sequences):
  * Only the valid prefix of each sample row touches the device, packed
    densely per core (transposed: h%128 on partitions, per slot 6 h-chunks
    x width contiguous), in FP16 -- halving HBM traffic vs fp32. The 2e-2
    harness gate leaves ~100x margin for fp16 rounding.
  * Rows (kind, sample) are distributed round-robin by length rank across
    the 8 cores -> identical SPMD program, balanced load.
  * Pooling runs on the DVE as tensor-tensor max folds
    (tensor_tensor with op=max), which in 16-bit
    get the 2x_1p perf mode (2 elem/lane/cyc) -- unlike tensor_reduce
    which is stuck at 1x. Each fold halves a group's width with an
    overlapped (idempotent, max) split so every level keeps 4B alignment
    with only pad-to-multiple-of-4.
  * Folds stop at width 16 writing into a persistent staging tile
    [128, 128slots*6ch, 16]; shared folds (16->8->4->2->1) finish 32
    slots at a time, then the tiny linear layer runs on the tensor
    engine in fp16 (K=128 per chunk, PSUM f32).
  * Groups (equal padded width) are planned by a small DP balancing DMA
    padding vs DVE instruction count; DMA tiles ramp up at the start and
    taper at the end so the vector engine starts early and drains fast.
"""

import numpy as np

B, S, H, C = 512, 256, 768, 2
NCORES = 8
CH = H // 128            # 6 h-chunks of 128 partitions
KINDS = 2
SLOTS = B // NCORES      # 64 slots per kind per core
SLOT_TOT = KINDS * SLOTS
NEG = np.float16(-60000.0)

TILE_W = 20480           # fp16 elems per partition per SBUF data tile
DATA_BUFS = 4
SEG_RAMP = [512, 1024, 2048, 4096, 6144]
SEG_W = 9216             # steady-state dma segment width

# planning cost model
DMA_NS_PER_COL = 1536.0 / 410.0   # one slot-col = 6 elems x 128 part x 2B
DVE_GHZ = 0.96
OP_OVERHEAD_CYC = 105.0  # measured ~110ns/op incl drain
LAMBDAS = [0.5, 1.0, 1.5, 2.0, 3.0, 4.0, 6.0, 8.0, 12.0]
STW = 16                 # staging width: fold chains stop here


def _eff_lengths(m):
    am = np.argmin(np.asarray(m), axis=1)
    return np.where(am == 0, S, am).astype(np.int64)


def _ceil4(x):
    return (int(x) + 3) // 4 * 4


def _chain(W):
    """Fold chain for a group of width W (mult of 4, >= 16).

    Returns (n_ops, out_elems_per_row): in-place halving folds down to
    <= 32 wide, then one fold to width 16 (into staging); the shared
    epilogue finishes 16 -> 1."""
    ops, outs, c = 0, 0, int(W)
    while c > 2 * STW:
        h = _ceil4((c + 1) // 2)
        ops += 1
        outs += h
        c = h
    ops += 1
    outs += STW
    return ops, outs


def _plan_widths(widths, lam):
    """Group descending padded widths into equal-width groups via DP.

    Cost per group: DMA of n*W slot-cols + lam * per-op overhead.
    Constraint: n*CH*W <= TILE_W. Returns final width per slot."""
    n = len(widths)
    INF = float("inf")
    best = [INF] * (n + 1)
    best[0] = 0.0
    prev = [0] * (n + 1)
    for i in range(1, n + 1):
        j = i - 1
        while j >= 0:
            Wg = widths[j]
            if (i - j) * CH * Wg > TILE_W:
                break
            ops, _ = _chain(Wg)
            cost = (
                best[j]
                + (i - j) * Wg * DMA_NS_PER_COL
                + lam * ops * OP_OVERHEAD_CYC / DVE_GHZ
            )
            if cost < best[i]:
                best[i] = cost
                prev[i] = j
            j -= 1
    out = np.zeros(n, dtype=np.int64)
    i = n
    while i > 0:
        j = prev[i]
        out[j:i] = widths[j]
        i = j
    return out


def _estimate(final_widths_by_kind):
    """(dma_ns, dve_ns) rough estimates for a plan."""
    cols = 0
    work_cyc = 0.0
    ops = 0
    for fw in final_widths_by_kind:
        i = 0
        n = len(fw)
        while i < n:
            j = i
            while j < n and fw[j] == fw[i] and (j - i + 1) * CH * fw[i] <= TILE_W:
                j += 1
            g = j - i
            c_ops, c_outs = _chain(fw[i])
            cols += g * fw[i]
            work_cyc += CH * g * c_outs / 2.0
            ops += c_ops
            i = j
    dma_ns = cols * DMA_NS_PER_COL
    dve_ns = (work_cyc + ops * OP_OVERHEAD_CYC + 9000.0) / DVE_GHZ
    return dma_ns, dve_ns


def _make_plan(slot_w):
    """slot_w: [kind][64] true widths desc. Returns final padded widths."""
    padded = [np.maximum([_ceil4(w) for w in sw], STW) for sw in slot_w]
    best = None
    for lam in LAMBDAS:
        fw = [_plan_widths(p, lam) for p in padded]
        dma, dve = _estimate(fw)
        # wall model: DVE path (starts ~10.5us after t0) vs DMA path with
        # ~19% slow-engine margin (engine 15 on unlucky cores)
        obj = (max(dve + 10500.0, dma * 1.19 + 8500.0), dma)
        if best is None or obj < best[0]:
            best = (obj, fw)
    return best[1]


def _layout(final_w):
    """Row-granular layout from final slot widths.

    Each slot is 6 chunk-rows of width W, contiguous in DRAM columns and
    in staging rows. Returns (R, tiles); tiles = [(c0, c1, segments)],
    segments = [(s0, s1, [(a, row0, nrows, W), ...])] where a is the
    elem offset of the fold-group, row0 its first staging row.
    """
    row_w = []  # width per staging row, in emission order
    for k in range(KINDS):
        for i in range(SLOTS):
            row_w.extend([int(final_w[k][i])] * CH)
    total = sum(row_w)
    offs = [np.zeros(SLOTS, dtype=np.int64) for _ in range(KINDS)]
    off = 0
    for k in range(KINDS):
        for i in range(SLOTS):
            offs[k][i] = off
            off += CH * int(final_w[k][i])

    # segment schedule: ramp up, steady, taper down
    seg_caps = []
    done = 0
    nrows = len(row_w)
    r = 0
    segs_flat = []  # (elem_start, elem_end, row_start, row_end)
    off = 0
    while r < nrows:
        rem = total - done
        nseg = len(segs_flat)
        if nseg < len(SEG_RAMP):
            cap = SEG_RAMP[nseg]
        elif rem <= 12288:
            cap = max(2048, rem // 3 + 256)
        else:
            cap = SEG_W
        e0 = off
        r0 = r
        cur = 0
        while r < nrows and (cur == 0 or cur + row_w[r] <= cap):
            cur += row_w[r]
            off += row_w[r]
            r += 1
        done += cur
        segs_flat.append((e0, off, r0, r))

    # pack segments into data tiles (SBUF buffers) of <= TILE_W elems
    tiles = []
    cur_segs = []
    for (e0, e1, r0, r1) in segs_flat:
        if cur_segs and e1 - cur_segs[0][0] > TILE_W:
            tiles.append(cur_segs)
            cur_segs = []
        cur_segs.append((e0, e1, r0, r1))
    if cur_segs:
        tiles.append(cur_segs)

    out_tiles = []
    for segs in tiles:
        c0 = segs[0][0]
        c1 = segs[-1][1]
        out_segs = []
        for (e0, e1, r0, r1) in segs:
            groups = []
            a = e0
            for r in range(r0, r1):
                if groups and groups[-1][3] == row_w[r]:
                    groups[-1][2] += 1
                else:
                    groups.append([a, r, 1, row_w[r]])
                a += row_w[r]
            out_segs.append((e0, e1, tuple(tuple(g) for g in groups)))
        out_tiles.append((c0, c1, tuple(out_segs)))
    return total, tuple(out_tiles), offs


def _build_program(R, tiles):
    import concourse.bacc as bacc
    import concourse.mybir as mybir
    from concourse.tile import TileContext

    f16 = mybir.dt.float16
    f32 = mybir.dt.float32
    MAX = mybir.AluOpType.max

    # last tile index touching each 16-slot staging chunk, for epilogues
    NHALF = 8
    HROWS = SLOT_TOT * CH // NHALF
    last_tile_of_half = [0] * NHALF
    for ti, (_, _, segs) in enumerate(tiles):
        for (_, _, groups) in segs:
            for (_, r0, nr, _) in groups:
                for h in range(r0 // HROWS, (r0 + nr - 1) // HROWS + 1):
                    last_tile_of_half[h] = ti

    nc = bacc.Bacc("TRN2", target_bir_lowering=False, debug=False, num_devices=NCORES)
    p_in = nc.dram_tensor("p", [128, R], f16, kind="ExternalInput")
    wt_in = nc.dram_tensor("wt", [128, KINDS * CH, C], f16, kind="ExternalInput")
    out_d = nc.dram_tensor("out", [C, SLOT_TOT], f32, kind="ExternalOutput")

    with TileContext(nc) as tc:
        with (
            tc.tile_pool(name="data", bufs=DATA_BUFS) as data_pool,
            tc.tile_pool(name="stage", bufs=1) as stage_pool,
            tc.tile_pool(name="small", bufs=1) as small_pool,
            tc.tile_pool(name="psum", bufs=1, space="PSUM") as psum_pool,
        ):
            wt_t = small_pool.tile([128, KINDS * CH, C], f16, tag="wt")
            stage = stage_pool.tile([128, SLOT_TOT * CH, STW], f16, tag="stage")
            out_sb = small_pool.tile([C, SLOT_TOT], f32, tag="osb")

            def fold(view, w, h, out_view=None):
                # out = max(view[:, :, :h], view[:, :, w-h:w]) (overlap ok)
                nc.vector.tensor_tensor(
                    out=view[:, :, :h] if out_view is None else out_view,
                    in0=view[:, :, :h],
                    in1=view[:, :, w - h : w],
                    op=MAX,
                )

            CSLOTS = SLOT_TOT // NHALF  # 16 slots per epilogue chunk
            ps_tiles = [None, None]

            def epilogue_half(hf):
                rows = stage[:, hf * HROWS : (hf + 1) * HROWS, :]
                w = STW
                while w > 1:
                    fold(rows, w, w // 2)
                    w //= 2
                # per-chunk matmul + psum->sbuf copy (PE/ACT are idle)
                k = hf // (NHALF // KINDS)
                if ps_tiles[k] is None:
                    ps_tiles[k] = psum_pool.tile([C, SLOTS], f32, tag=f"ps{k}", name=f"ps{k}")
                ps = ps_tiles[k]
                srg = stage[:, :, :].rearrange("p (s c) e -> p s c e", c=CH)
                sl0 = (hf % (NHALF // KINDS)) * CSLOTS
                for ch in range(CH):
                    nc.tensor.matmul(
                        ps[:, sl0 : sl0 + CSLOTS],
                        lhsT=wt_t[:, k * CH + ch, :],
                        rhs=srg[:, k * SLOTS + sl0 : k * SLOTS + sl0 + CSLOTS, ch, 0],
                        start=(ch == 0),
                        stop=(ch == CH - 1),
                    )
                nc.scalar.copy(
                    out=out_sb[:, k * SLOTS + sl0 : k * SLOTS + sl0 + CSLOTS],
                    in_=ps[:, sl0 : sl0 + CSLOTS],
                )
                if hf % (NHALF // KINDS) == NHALF // KINDS - 1:
                    nc.sync.dma_start(
                        out=out_d[:, k * SLOTS : (k + 1) * SLOTS],
                        in_=out_sb[:, k * SLOTS : (k + 1) * SLOTS],
                    )

            nseg_done = 0
            for ti, (c0, c1, segs) in enumerate(tiles):
                dt = data_pool.tile([128, TILE_W], f16, tag="data")
                for (e0, e1, groups) in segs:
                    nc.sync.dma_start(
                        out=dt[:, e0 - c0 : e1 - c0], in_=p_in[:, e0:e1]
                    )
                    nseg_done += 1
                    if nseg_done == 2:
                        nc.scalar.dma_start(out=wt_t, in_=wt_in[:, :, :])
                    for (a, r0, nr, W) in groups:
                        base = a - c0
                        view = dt[:, base : base + nr * W].rearrange(
                            "p (r w) -> p r w", w=W
                        )
                        w = W
                        while w > 2 * STW:
                            h = _ceil4((w + 1) // 2)
                            fold(view, w, h)
                            w = h
                        st = stage[:, r0 : r0 + nr, :]
                        fold(view, w, STW, out_view=st)
                for hf in range(NHALF):
                    if last_tile_of_half[hf] == ti:
                        epilogue_half(hf)

    nc.compile()
    return nc


_NC_CACHE = {}


def kernel(x1, x2, m1, m2, W, b, _run_opts=None):
    from concourse.bass_utils import run_bass_kernel_spmd

    x1 = np.asarray(x1)
    x2 = np.asarray(x2)
    W32 = np.asarray(W, dtype=np.float32)
    b32 = np.asarray(b, dtype=np.float32)
    effs = [_eff_lengths(m1), _eff_lengths(m2)]
    orders = [np.argsort(-effs[k], kind="stable") for k in range(KINDS)]
    slot_w = [effs[k][orders[k][:: NCORES]].astype(np.int64) for k in range(KINDS)]

    final_w = _make_plan(slot_w)
    R, tiles, offs = _layout(final_w)

    key = (R, tuple(tiles))
    nc = _NC_CACHE.get(key)
    if nc is None:
        nc = _build_program(R, tiles)
        _NC_CACHE[key] = nc

    # pack per-core fp16 data
    xh = [x1.astype(np.float16), x2.astype(np.float16)]
    packs = np.full((NCORES, 128, R), NEG, dtype=np.float16)
    for k in range(KINDS):
        eff, order = effs[k], orders[k]
        xk = xh[k]
        for i in range(SLOTS):
            off = int(offs[k][i])
            v = int(final_w[k][i])
            for c in range(NCORES):
                bidx = order[i * NCORES + c]
                e = int(eff[bidx])
                dst = packs[c][:, off : off + CH * v].reshape(128, CH, v)
                dst[:, :, :e] = xk[bidx, :e, :].reshape(e, CH, 128).transpose(2, 1, 0)

    wtp = np.ascontiguousarray(
        W32.astype(np.float16).reshape(C, KINDS, CH, 128).transpose(3, 1, 2, 0)
    ).reshape(128, KINDS * CH, C)

    in_maps = [{"p": packs[c], "wt": wtp} for c in range(NCORES)]

    res = None
    last_err = None
    for _attempt in range(3):
        try:
            res = run_bass_kernel_spmd(
                nc, in_maps, core_ids=list(range(NCORES)), **(_run_opts or {})
            )
            break
        except Exception as e:  # wedged device etc. -- retry
            last_err = e
    if res is None:
        raise last_err

    out_full = np.zeros((B, C), dtype=np.float32)
    res_all = np.stack([res.results[c]["out"] for c in range(NCORES)])  # [8, C, 128]
    for k in range(KINDS):
        part = res_all[:, :, k * SLOTS : (k + 1) * SLOTS]  # [core, C, slot]
        part = part.transpose(2, 0, 1).reshape(B, C)  # [(slot, core), C]
        out_full[orders[k]] += part
    out_full += b32[None, :]
    if _run_opts is not None:
        kernel._last_res = res
    return out_full
